# revision 10
# baseline (speedup 1.0000x reference)
"""Trainium2 Bass kernel for CoExDispProcessor (topk_masking).

Per-sample computation (data-parallel over batch across 8 cores):
  1. top-2 over the D=48 disparity axis of cost [1,48,128,240] -> softmax
     blend of the two indices -> disp4 [128,240]
  2. 3x3 unfold of disp4 (zero pad) -> nearest 4x upsample -> weighted sum
     with softmax over the 9 channels of spg [9,512,960] -> disp1 [512,960]

v4 design:
  - cost TAGGED on host: low 2 mantissa bits := (w mod 4) (<=2 ulp noise,
    far below the fine stage's fp16).  Any 4 consecutive columns are then
    pairwise distinct, so ONE max_index serves a 4-column window (scan
    d-major: position p = 4*d + c_rel, exactly recoverable).  60 window
    max_index calls replace 240 per-column ones.
  - max8 writes PLANE-MAJOR via a scattered [[240,8]] output AP: ranks in
    v8p[128, 8, 240]; v1/v2 planes contiguous (fast dv), one GPSIMD copy
    packs (v1,v2) pairs into v2c[128, 240, 2] for contiguous in_max.
  - coarse in 4 column quarters; fine chunks interleaved between quarters
    on DVE so the GPSIMD shift-DMA completion latency is hidden.
  - ACT queue: exp groups / t-exp-q / urep-q interleaved by readiness; one
    Reciprocal at the end (2 table loads total).
  - fine: per chunk one e-tile [128,9,4,240] fp16; products + 9->1 tree
    IN-PLACE on DVE; den tree on GPSIMD; r0 = 4/den via Recip scale=0.25
    in-place in den_all.
  - out fp16 staging tile, single DMA; host converts to f32.
"""

import os
import sys
from contextlib import ExitStack

import numpy as np

if "/opt/trn_rl_repo" not in sys.path:
    sys.path.insert(0, "/opt/trn_rl_repo")

import concourse.bass as bass
import concourse.bacc as bacc
import concourse.tile as tile
from concourse import mybir
from concourse.ap import AP
from concourse.bass_utils import run_bass_kernel_spmd

F32 = mybir.dt.float32
FP16 = mybir.dt.float16
U16 = mybir.dt.uint16
OP = mybir.AluOpType
ACT = mybir.ActivationFunctionType

B, D, H, W = 8, 48, 128, 240
HF, WF = 4 * H, 4 * W  # 512, 960
N_CORES = 8

NK = 4               # fine column chunks / coarse quarters
WCF = WF // NK       # fine cols per chunk (240)
QB = [0, 64, 128, 192, 240]  # coarse quarter boundaries (multiples of 4)
NW = W // 4          # max_index windows (60)


def _act_raw(nc, out_ap, in_ap, func, scale=1.0, bias=0.0):
    """Raw InstActivation (bypasses the Reciprocal accuracy guard)."""
    eng = nc.scalar
    return eng.add_instruction(
        mybir.InstActivation(
            name=nc.get_next_instruction_name(),
            func=func,
            ins=[
                eng.lower_ap(in_ap),
                mybir.ImmediateValue(dtype=F32, value=bias),
                mybir.ImmediateValue(dtype=F32, value=scale),
                mybir.ImmediateValue(dtype=F32, value=0.0),
            ],
            outs=[eng.lower_ap(out_ap)],
        )
    )


def _max8_planar(nc, v8p, ctile, j):
    """max8 of column j with ranks scattered to planes: v8p[:, r, j]."""
    base = v8p[:]
    out_ap = AP(base.tensor, base.offset + j, [list(base.ap[0]), [W, 8]])
    eng = nc.vector
    return eng.add_instruction(
        mybir.InstMax(
            name=nc.get_next_instruction_name(),
            ins=[eng.lower_ap(ctile[:, :, j])],
            outs=[eng.lower_ap(out_ap)],
        )
    )


def _max_index_raw(nc, out_ap, in_max_ap, in_values_ap):
    """InstMaxIndex without the 2D-shape asserts (multi-column window)."""
    eng = nc.vector
    return eng.add_instruction(
        mybir.InstMaxIndex(
            name=nc.get_next_instruction_name(),
            ins=[eng.lower_ap(in_max_ap), eng.lower_ap(in_values_ap)],
            outs=[eng.lower_ap(out_ap)],
        )
    )


def _strided(base_ap, off_elems, pairs):
    """New AP on the same tile: keep partition dim, custom free dims."""
    return AP(base_ap.tensor, base_ap.offset + off_elems,
              [list(base_ap.ap[0])] + [list(p) for p in pairs])


def build_kernel(ctx: ExitStack, tc: tile.TileContext, out_d, cost_d, spg_d):
    nc = tc.nc

    cost_hdw = cost_d.transpose([1, 0, 2])  # [128(h), 48(d), 240(w)] view
    spg_v = spg_d.rearrange("c (R dr) (k w) -> c R dr k w", dr=4, k=NK)
    out_v = out_d.rearrange("(R dr) x -> R dr x", dr=4)  # [128, 4, 960]

    costp = ctx.enter_context(tc.tile_pool(name="costp", bufs=1))
    rawp = ctx.enter_context(tc.tile_pool(name="rawp", bufs=3))
    ep = ctx.enter_context(tc.tile_pool(name="ep", bufs=NK))
    dstagep = ctx.enter_context(tc.tile_pool(name="dstagep", bufs=1))
    persist = ctx.enter_context(tc.tile_pool(name="persist", bufs=1))

    # ---- input DMAs on the sync (SP) queue: cost first, then spg --------
    ctile = costp.tile([128, D, W], F32, tag="cost")
    nc.sync.dma_start(ctile[:], cost_hdw)

    raws = {}
    for k in range(NK):
        for g in range(3):  # channel groups of 3 (one DMA per channel)
            r = rawp.tile([128, 3, 4, WCF], F32, tag="raw")
            for i in range(3):
                nc.sync.dma_start(r[:, i], spg_v[3 * g + i, :, :, k, :])
            raws[(k, g)] = r

    # ---- persistent tiles ------------------------------------------------
    v8p = persist.tile([128, 8, W], F32)       # rank-major top-8 planes
    v2c = persist.tile([128, W, 2], F32)       # (v1, v2) pair-major
    pos8 = persist.tile([128, NW, 8], U16)
    i1pf = persist.tile([128, W], F32)
    i2pf = persist.tile([128, W], F32)
    i1f = persist.tile([128, W], F32)
    i2f = persist.tile([128, W], F32)
    dv = persist.tile([128, W], F32)
    tt = persist.tile([128, W], F32)
    denc = persist.tile([128, W], F32)
    rden = persist.tile([128, W], F32)
    numc = persist.tile([128, W], F32)
    ctab = persist.tile([128, 4], F32)
    rvall = persist.tile([128, 3, W + 2], F32)
    urep = persist.tile([128, 3, 4 * (W + 2)], FP16)
    den_all = persist.tile([128, NK, 4, WCF], FP16)
    outstage = persist.tile([128, 4, WF], FP16)

    nc.vector.memset(rvall[:], 0.0)
    for c in range(4):
        nc.gpsimd.memset(ctab[:, c:c + 1], c * 0.25)

    e_tiles = {}

    def exps(k):
        e = ep.tile([128, 9, 4, WCF], FP16, tag="e")
        e_tiles[k] = e
        for g in range(3):
            nc.scalar.activation(e[:, 3 * g:3 * g + 3], raws[(k, g)][:], ACT.Exp)

    def den_gp(k):
        e = e_tiles[k]
        s = dstagep.tile([128, 4, 4, WCF], FP16, tag="dstage")
        nc.gpsimd.tensor_add(s[:], e[:, 0:4], e[:, 4:8])
        nc.gpsimd.tensor_add(s[:, 0:2], s[:, 0:2], s[:, 2:4])
        nc.gpsimd.tensor_add(s[:, 0], s[:, 0], s[:, 1])
        nc.gpsimd.tensor_add(den_all[:, k], s[:, 0], e[:, 8])

    def coarse_q(q):
        """DVE: max8 + windowed idx + recovery + dv for quarter q.
        GP: pair-pack + position casts."""
        a, b = QB[q], QB[q + 1]
        nq = b - a
        for j in range(a, b):
            _max8_planar(nc, v8p, ctile, j)
        # pack (v1,v2) pairs for contiguous in_max (GPSIMD, strided out ok)
        nc.gpsimd.tensor_copy(
            v2c[:, a:b].rearrange("p w r -> p r w"), v8p[:, 0:2, a:b]
        )
        for m in range(a // 4, b // 4):
            _max_index_raw(
                nc, pos8[:, m],
                v2c[:, 4 * m:4 * m + 4].rearrange("p a b -> p (a b)"),
                ctile[:, :, 4 * m:4 * m + 4],
            )
        # u16 position -> f32 (GPSIMD)
        p8 = pos8[:]
        nc.gpsimd.tensor_copy(i1pf[:, a:b], _strided(p8, 2 * a, [[2, nq]]))
        nc.gpsimd.tensor_copy(i2pf[:, a:b], _strided(p8, 2 * a + 1, [[2, nq]]))
        # d = p/4 - (w mod 4)/4   (DVE)
        cb = ctab[:].unsqueeze(1).broadcast_to([128, nq // 4, 4])
        for pf, ifl in ((i1pf, i1f), (i2pf, i2f)):
            nc.vector.scalar_tensor_tensor(
                ifl[:, a:b].rearrange("p (m f) -> p m f", f=4),
                pf[:, a:b].rearrange("p (m f) -> p m f", f=4),
                0.25, cb, op0=OP.mult, op1=OP.subtract,
            )
        nc.vector.tensor_sub(dv[:, a:b], v8p[:, 1, a:b], v8p[:, 0, a:b])

    def blend_q(q):
        """DVE: disp4 = (i1 + t*i2)/(1+t) for quarter q (needs tt[q])."""
        a, b = QB[q], QB[q + 1]
        nc.vector.tensor_scalar_add(denc[:, a:b], tt[:, a:b], 1.0)
        nc.vector.reciprocal(rden[:, a:b], denc[:, a:b])
        nc.vector.tensor_mul(numc[:, a:b], tt[:, a:b], i2f[:, a:b])
        nc.vector.tensor_add(numc[:, a:b], numc[:, a:b], i1f[:, a:b])
        nc.vector.tensor_mul(rvall[:, 1, 1 + a:1 + b], numc[:, a:b],
                             rden[:, a:b])

    def shifts_q(q):
        """GPSIMD SWDGE row-shift DMAs for quarter q's columns."""
        a, b = QB[q], QB[q + 1]
        nc.gpsimd.dma_start(rvall[1:128, 0, 1 + a:1 + b],
                            rvall[0:127, 1, 1 + a:1 + b])
        nc.gpsimd.dma_start(rvall[0:127, 2, 1 + a:1 + b],
                            rvall[1:128, 1, 1 + a:1 + b])

    def texp_q(q):
        a, b = QB[q], QB[q + 1]
        nc.scalar.activation(tt[:, a:b], dv[:, a:b], ACT.Exp)

    def urep_q(q):
        """ACT: 4x replicate rv cols [64q, 64q+64) (q3: through 242)."""
        a = QB[q]
        b = QB[q + 1] if q < 3 else W + 2
        ncols = b - a
        nc.scalar.copy(
            urep[:, :, 4 * a:4 * b].rearrange("p s (c f) -> p s c f", f=4),
            rvall[:, :, a:b].unsqueeze(3).broadcast_to([128, 3, ncols, 4]),
        )

    def fine(k):
        e = e_tiles[k]
        for c in range(9):
            ci, cj = c // 3, c % 3
            off = WCF * k + 4 * cj
            u4 = (urep[:, ci, off:off + WCF]
                  .unsqueeze(1).broadcast_to([128, 4, WCF]))
            nc.vector.tensor_mul(e[:, c], e[:, c], u4)
        nc.vector.tensor_add(e[:, 0:4], e[:, 0:4], e[:, 4:8])
        nc.vector.tensor_add(e[:, 0:2], e[:, 0:2], e[:, 2:4])
        nc.vector.tensor_add(e[:, 0], e[:, 0], e[:, 1])
        nc.vector.tensor_add(e[:, 0], e[:, 0], e[:, 8])  # num in e[:, 0]

    # ---- schedule --------------------------------------------------------
    # ACT queue: e-k0, t-q0, e-k1, t-q1, urep-q0, e-k2, t-q2, urep-q1,
    #            e-k3, t-q3, urep-q2, urep-q3, recip
    # DVE queue: q0, b0, q1, b1, fk0, q2, b2, fk1, q3, b3, fk2, fk3, finals
    # GP  queue: ctab, [pack/casts inside coarse_q], shifts-q after blends,
    #            den-k0..k3
    exps(0)
    coarse_q(0)
    texp_q(0)
    blend_q(0)
    shifts_q(0)
    exps(1)
    den_gp(0)
    coarse_q(1)
    texp_q(1)
    blend_q(1)
    shifts_q(1)
    urep_q(0)
    fine(0)
    exps(2)
    den_gp(1)
    coarse_q(2)
    texp_q(2)
    blend_q(2)
    shifts_q(2)
    urep_q(1)
    fine(1)
    exps(3)
    den_gp(2)
    coarse_q(3)
    texp_q(3)
    blend_q(3)
    shifts_q(3)
    den_gp(3)
    urep_q(2)
    urep_q(3)
    fine(2)
    fine(3)
    _act_raw(nc, den_all[:], den_all[:], ACT.Reciprocal, scale=0.25)  # 4/den
    for k in range(NK):
        nc.vector.tensor_mul(
            outstage[:, :, WCF * k:WCF * (k + 1)], e_tiles[k][:, 0],
            den_all[:, k],
        )
    nc.sync.dma_start(out_v, outstage[:])


def build_program():
    nc = bacc.Bacc(
        "TRN2",
        target_bir_lowering=False,
        debug=False,
        enable_asserts=False,
        num_devices=N_CORES,
    )
    cost_d = nc.dram_tensor("cost", [D, H, W], F32, kind="ExternalInput").ap()
    spg_d = nc.dram_tensor("spg", [9, HF, WF], F32, kind="ExternalInput").ap()
    out_d = nc.dram_tensor("out", [HF, WF], FP16, kind="ExternalOutput").ap()
    with tile.TileContext(nc) as tc:
        with ExitStack() as ctx:
            build_kernel(ctx, tc, out_d, cost_d, spg_d)
    nc.compile()
    return nc


def _install_ntff_hook():
    """Provide antenv.axon_hooks + register the ctypes NTFF profiler."""
    import types

    if "antenv.axon_hooks" in sys.modules:
        return True
    try:
        import antenv
        from trn_agent_boot.trn_boot import _ntff_profile_via_ctypes

        mod = types.ModuleType("antenv.axon_hooks")
        mod._hook = None

        def set_axon_ntff_profile_hook(hook):
            mod._hook = hook

        def get_axon_ntff_profile_hook():
            return mod._hook

        mod.set_axon_ntff_profile_hook = set_axon_ntff_profile_hook
        mod.get_axon_ntff_profile_hook = get_axon_ntff_profile_hook
        sys.modules["antenv.axon_hooks"] = mod
        antenv.axon_hooks = mod
        mod._hook = _ntff_profile_via_ctypes("/opt/axon/libaxon_pjrt.so")
        return True
    except Exception as e:  # profiling is best-effort
        print(f"NTFF hook install failed: {e}")
        return False


LAST_RESULTS = None


def kernel(cost: np.ndarray, spg: np.ndarray) -> np.ndarray:
    """cost [8,1,48,128,240] f32, spg [8,9,512,960] f32 -> disp1 [8,512,960] f32."""
    global LAST_RESULTS
    cost = np.ascontiguousarray(np.asarray(cost, dtype=np.float32))
    spg = np.ascontiguousarray(np.asarray(spg, dtype=np.float32))
    assert cost.shape == (B, 1, D, H, W) and spg.shape == (B, 9, HF, WF)

    # Tag cost: low 2 mantissa bits := (w mod 4). <= 2 ulp perturbation;
    # makes any 4 consecutive columns pairwise distinct so the windowed
    # max_index cannot cross-match between columns.
    u = cost[:, 0].view(np.uint32)
    tag = (np.arange(W, dtype=np.uint32) & np.uint32(3))
    cost_t = ((u & ~np.uint32(3)) | tag[None, None, None, :]).view(np.float32)

    nc = build_program()
    in_maps = [
        {"cost": cost_t[b], "spg": spg[b]} for b in range(B)
    ]
    trace = bool(int(os.environ.get("KERNEL_TRACE", "0")))
    if trace:
        trace = _install_ntff_hook()
    res = run_bass_kernel_spmd(
        nc, in_maps, core_ids=list(range(N_CORES)), trace=trace
    )
    LAST_RESULTS = res
    out = np.stack([res.results[b]["out"] for b in range(B)], axis=0)
    return out.astype(np.float32, copy=False)
